# revision 32
# baseline (speedup 1.0000x reference)
"""Causal self-attention Trainium2 Bass kernel.

Problem: B=16, T=2048, D=128, H=4 (head dim 32), fp32 in/out.
  qkv = x @ w_attn ; per-head scores = q k^T / sqrt(32), causal
  y = softmax(scores) @ v ; out = y @ w_proj

Sharding: data-parallel over batch, 2 batches per NeuronCore x 8 cores
(SPMD, no collectives).

Per-core pipeline (measured 412 us/core on TRN2 vs 810 us for the pure
fp32 variant; flip CONFIG["attn_dt"] to "f32" for bit-conservative mode):

  phase A: x tiles DMA'd in 512-token blocks, PE-transposed -> xT [128d, T]
    (fp32); QKV^T projection (fp32): Q^T/K^T stay head-major [128ch, T]
    (head h lives on partitions 32h:32h+32 = exactly the K=32 contraction
    layout the scores matmuls need); V is produced token-major and stored
    per head as V' [128k, 128] = [V_h | ones | zeros] in fp16 - the ones
    column makes the attn@V matmul emit the softmax denominator row for
    free (row 32 for even heads, 64 for odd heads).

  phase B, per (batch, 512-wide q-group, head-pair), fp16 operands:
    for each causal k-chunk (128 wide):
      scores^T [128k, 2h, 512q] = two K=32 matmuls packed into the PE
        array with tile_position=(32h, 0) row tiling
      diagonal-band chunks: -1e9 additive mask on the 128x128 diagonal
        block (DVE), exp skips the fully-masked region left of it
      exp on ScalarE straight out of the 2-bank PSUM quad (scale fused,
        fp16 out), double-buffered quads keep ACT saturated
      attn@V: per head one M=128 matmul (fp16) accumulating y^T + sums
        into that head's PSUM bank
    normalize: sums rows staged into a zeroed SBUF tile (partition-aligned
      copies), one expander matmul broadcasts both heads' sums to
      partitions 0:64, DVE reciprocal + two multiplies -> ysbP (even head
      rows 0:32, odd rows 32:64, rest kept zero)
    first 128 query rows are recomputed exactly in fp32 (they attend over
      few keys, so fp16 rounding is not averaged away) and overwrite the
      fp16 result in y columns 0:128 before normalization
    out projection: two accumulating K=128 fp32 matmuls per 128-token
      chunk against per-pair row-packed w_proj tiles; 512-token batched
      DMA out.

Hardware traps encountered (avoid on TRN2):
  - M<128 col-tiled matmuls (tile_position=(0,64), M=33) crash the device
    for fp32 and fp16 accumulation breaks silently at M=64; all attn@V
    matmuls here use M=128.
  - Accumulating K=32 matmuls at *different* row positions into one PSUM
    bank crashes; same-position accumulation is fine.
  - float32r needs DVE-produced operands (special storage format), and is
    invalid ISA for col-offset tile positions; fp16 is the usable fast
    dtype (ACT can write it directly).
  - fp32 matmul costs 4 cycles/row (two half-speed passes); fp16 is 1.
  - Engine partition offsets must be 32-aligned; cross-operand partition
    shifts in 32-steps are legal on DVE.
"""

import os
import sys

import numpy as np

for _p in ("/opt/trn_rl_repo", "/root/.axon_site/_ro/trn_rl_repo"):
    if os.path.isdir(_p) and _p not in sys.path:
        sys.path.insert(0, _p)

import concourse.bass as bass
import concourse.bacc as bacc
import concourse.mybir as mybir
import concourse.tile as tile
from concourse.bass_utils import run_bass_kernel_spmd

F32 = mybir.dt.float32
P = 128
NEG = -1.0e9

# full problem shape (hardcoded per harness contract)
B, T, D, H = 16, 2048, 128, 4
DH = D // H  # 32
N_CORES = 8
BPC = B // N_CORES  # batches per core


def build_attention_nc(bpc=BPC, t=T, qg=512, loop_n=0, attn_dt="f32",
                       quad_bufs=1, exp_bufs=3, av_f32=False,
                       trim=True, postmask=True, fp16a=True, fp16np=True,
                       exp_mode="3d", et_dt=None, ablate=""):
    """Build the single-core SPMD Bass program.

    bpc: batches this core handles; t: sequence length; qg: q-group width.
    trim: narrow the diag-band matmuls/exp to the live q range.
    postmask: mask diag block by fp16 multiply after exp (vs add before).
    fp16a: phase A (x transpose + QKV projection) in fp16 operands.
    fp16np: normalize expander + out projection in fp16 operands.
    """
    assert t % qg == 0 and qg % P == 0 and t % P == 0
    nqg = t // qg      # q groups
    nkc = t // P       # 128-wide k chunks
    cpq = qg // P      # k chunks per q group
    scale = 1.0 / float(np.sqrt(DH))
    # attention matmul operand dtype: fp16 runs the PE at 1 cycle/row
    # (vs 4 for fp32); softmax statistics strongly attenuate the rounding
    FA = {"f32": F32, "f16": mybir.dt.float16,
          "f32r": mybir.dt.float32r}[attn_dt]
    FV = F32 if av_f32 else FA  # attn@V operand dtype (exp tiles + V')
    if et_dt == "bf16":
        FV = mybir.dt.bfloat16
    # dependency-preserving ablations for timing attribution: shrink the
    # named op's free width to 64 columns (results become garbage)
    abl = set(a for a in ablate.split(",") if a)

    FP = mybir.dt.float16 if fp16a else F32   # phase-A operand dtype
    FN = mybir.dt.float16 if fp16np else F32  # normalize/out-proj dtype

    nc = bacc.Bacc("TRN2", target_bir_lowering=False, debug=False)
    x_d = nc.dram_tensor("x", [bpc, t, D], F32, kind="ExternalInput")
    wa_d = nc.dram_tensor("w_attn", [D, 3 * D], F32, kind="ExternalInput")
    wp_d = nc.dram_tensor("w_proj", [D, D], F32, kind="ExternalInput")
    out_d = nc.dram_tensor("out", [bpc, t, D], F32, kind="ExternalOutput")

    with tile.TileContext(nc) as tc:
        with tc.tile_pool(name="resident", bufs=1) as res:
            # ---- constants ----
            wa_sb = res.tile([D, 3 * D], F32, name="wa", tag="wa")
            nc.sync.dma_start(wa_sb[:], wa_d[:])
            wa16 = res.tile([D, 3 * D], FP, name="wa16", tag="wa16")
            nc.vector.tensor_copy(wa16[:], wa_sb[:])
            # w_proj rows permuted to match where heads land in the y psum
            # tiles: pass A holds heads {0,1} at partitions {0:32, 64:96},
            # pass B heads {2,3} likewise.
            # per-pair w_proj tiles: rows 0:32 = even head's w_proj rows,
            # rows 32:64 = odd head's, rest zero (the out projection
            # contracts all 128 partitions; ysb rows 64:128 likewise zero)
            wpF = res.tile([D, D], F32, name="wpf", tag="wpf")
            nc.sync.dma_start(wpF[:], wp_d[:])
            wpP = [res.tile([D, D], FN, name=f"wp{pi}", tag=f"wp{pi}")
                   for pi in range(2)]
            for pi in range(2):
                nc.vector.memset(wpP[pi][:], 0.0)
                for ci in range(2):
                    h = 2 * pi + ci
                    nc.vector.tensor_copy(wpP[pi][DH * ci:DH * (ci + 1), :],
                                          wpF[h * DH:(h + 1) * DH, :])

            ident = res.tile([P, P], F32, name="ident", tag="ident")
            nc.gpsimd.memset(ident[:], 0.0)
            nc.gpsimd.affine_select(
                out=ident[:], in_=ident[:],
                compare_op=mybir.AluOpType.not_equal, fill=1.0,
                base=0, pattern=[[-1, P]], channel_multiplier=1,
            )

            # causal additive mask for the diagonal 128x128 block:
            # cmask[k, c] = 0 if c >= k else NEG
            cmask = res.tile([P, P], F32, name="cmask", tag="cmask")
            nc.gpsimd.memset(cmask[:], 0.0)
            nc.gpsimd.affine_select(
                out=cmask[:], in_=cmask[:],
                compare_op=mybir.AluOpType.is_ge, fill=NEG,
                base=0, pattern=[[1, P]], channel_multiplier=-1,
            )
            # multiplicative causal mask applied to exp() output on the
            # diagonal block: bmask[k, c] = 1 if c >= k else 0 (fp16 so the
            # DVE runs it in 2x mode)
            bmask = res.tile([P, P], FV, name="bmask", tag="bmask")
            nc.gpsimd.memset(bmask[:], 1.0)
            nc.gpsimd.affine_select(
                out=bmask[:], in_=bmask[:],
                compare_op=mybir.AluOpType.is_ge, fill=0.0,
                base=0, pattern=[[1, P]], channel_multiplier=-1,
            )

            # combined expander: maps the pair's sums rows (32 -> partitions
            # 0:32 for the even head, 64 -> 32:64 for the odd head)
            exp_e = res.tile([P, 2 * DH], FN, name="exp_e", tag="exp_e")
            nc.gpsimd.memset(exp_e[:], 0.0)
            nc.gpsimd.memset(exp_e[32:33, 0:DH], 1.0)
            nc.gpsimd.memset(exp_e[64:65, DH:2 * DH], 1.0)

            # sums staging tiles (zeros except rows 32 / 96, rewritten per
            # q-group; zero rows make the expander matmul contraction clean)
            s_p = [res.tile([P, qg], FN, name=f"s_p{pi}", tag=f"s_p{pi}")
                   for pi in range(2)]
            for pi in range(2):
                nc.vector.memset(s_p[pi][:], 0.0)
            # resident normalized-y tiles, one per pair (even head rows 0:32,
            # odd head rows 32:64); rows 64:128 stay zero so the out
            # projection can contract all 128 partitions
            ysbP = [res.tile([P, qg], FN, name=f"ysbp{pi}", tag=f"ysbp{pi}")
                    for pi in range(2)]
            for pi in range(2):
                nc.vector.memset(ysbP[pi][:], 0.0)

            # fp32 shadows of q/k (first 128 tokens) and V chunk 0: the
            # first 128 query rows attend over few keys, so fp16 rounding
            # is not averaged away there; they are recomputed exactly and
            # overwrite the fp16 result
            qT32 = [res.tile([P, P], F32, name=f"qT32_{b}", tag=f"qT32_{b}")
                    for b in range(bpc)]
            kT32 = [res.tile([P, P], F32, name=f"kT32_{b}", tag=f"kT32_{b}")
                    for b in range(bpc)]
            vP32 = [res.tile([P, H, P], F32, name=f"vP32_{b}", tag=f"vP32_{b}")
                    for b in range(bpc)]
            for b in range(bpc):
                nc.gpsimd.memset(vP32[b][:], 0.0)
                nc.gpsimd.memset(vP32[b][:, 0::2, 32:33], 1.0)
                nc.gpsimd.memset(vP32[b][:, 1::2, 64:65], 1.0)

            # ---- per-batch resident activations ----
            xT = [res.tile([P, t], FP, name=f"xT{b}", tag=f"xT{b}") for b in range(bpc)]
            qT = [res.tile([P, t], FA, name=f"qT{b}", tag=f"qT{b}") for b in range(bpc)]
            kT = [res.tile([P, t], FA, name=f"kT{b}", tag=f"kT{b}") for b in range(bpc)]
            # V' per (b, h): [128 kpos, nkc, 64] = [V_h | ones | zeros].
            # 64 wide because M=33 col-tiled matmuls crash the device
            # (NRT_EXEC_UNIT_UNRECOVERABLE); M=64 at positions (0,0)/(0,64)
            # is solid. Col 32 stays 1.0 (softmax denominator trick).
            vP = [res.tile([P, nkc, H, P], FV, name=f"vp{b}", tag=f"vp{b}")
                  for b in range(bpc)]
            for b in range(bpc):
                nc.gpsimd.memset(vP[b][:], 0.0)
                # ones column at 32 for even heads, 64 for odd heads (the
                # pair's two sums land on different psum rows so both can be
                # staged into one s tile with partition-aligned copies)
                nc.gpsimd.memset(vP[b][:, :, 0::2, 32:33], 1.0)
                nc.gpsimd.memset(vP[b][:, :, 1::2, 64:65], 1.0)

            # optional hardware repeat loop (timing measurements only)
            import contextlib
            loop_cm = (tc.For_i(0, loop_n, 1) if loop_n
                       else contextlib.nullcontext())
            with loop_cm:
              # ============== phase A: x^T and QKV^T ==============
              with (
                  tc.tile_pool(name="xin", bufs=8) as xin_pool,
                  tc.tile_pool(name="psA", bufs=2, space="PSUM") as psA,
                  tc.tile_pool(name="psQK", bufs=2, space="PSUM") as psQK,
              ):
                  for b in range(bpc):
                      for kc in range(nkc):
                          xi = xin_pool.tile([P, D], F32, name="xin", tag="xin")
                          nc.sync.dma_start(xi[:], x_d[b, kc * P:(kc + 1) * P, :])
                          pst = psA.tile([P, P], F32, name="pst", tag="pst")
                          nc.tensor.transpose(pst[:], xi[:], ident[:])
                          nc.vector.tensor_copy(
                              xT[b][:, kc * P:(kc + 1) * P], pst[:])
                      # Q^T / K^T head-major
                      for j in range(t // 512):
                          sl = slice(j * 512, (j + 1) * 512)
                          for wofs, dst in ((0, qT[b]), (D, kT[b])):
                              pq = psQK.tile([P, 512], F32, name="pq", tag="pq")
                              nc.tensor.matmul(
                                  pq[:], wa16[:, wofs:wofs + D], xT[b][:, sl],
                                  start=True, stop=True)
                              nc.vector.tensor_copy(dst[:, sl], pq[:])
                              if j == 0:
                                  dst32 = qT32[b] if wofs == 0 else kT32[b]
                                  nc.vector.tensor_copy(dst32[:],
                                                        pq[:, 0:P])
                      # V token-major, scattered into per-head V' tiles
                      for kc in range(nkc):
                          pv = psA.tile([P, P], F32, name="pv", tag="pv")
                          nc.tensor.matmul(
                              pv[:], xT[b][:, kc * P:(kc + 1) * P],
                              wa16[:, 2 * D:3 * D], start=True, stop=True)
                          nc.vector.tensor_copy(
                              vP[b][:, kc, :, 0:DH],
                              pv[:].rearrange("p (h d) -> p h d", h=H))
                          if kc == 0:
                              nc.vector.tensor_copy(
                                  vP32[b][:, :, 0:DH],
                                  pv[:].rearrange("p (h d) -> p h d", h=H))

              # ================= phase B: attention =================
              with (
                  tc.tile_pool(name="quad", bufs=quad_bufs, space="PSUM") as quad_pool,
                  tc.tile_pool(name="ypsum", bufs=2, space="PSUM") as y_pool,
                  tc.tile_pool(name="aux", bufs=1, space="PSUM") as aux_pool,
                  tc.tile_pool(name="expt", bufs=exp_bufs) as exp_pool,
                  tc.tile_pool(name="yT", bufs=2) as yt_pool,
                  tc.tile_pool(name="outsb", bufs=6) as out_pool,
                  tc.tile_pool(name="rsb", bufs=4) as r_pool,
              ):
                  for b in range(bpc):
                      for j in range(nqg):
                          qsl = slice(j * qg, (j + 1) * qg)
                          kmax = cpq * (j + 1) - 1
                          for pi in range(2):  # head pairs (0,1), (2,3)
                              y_p = [y_pool.tile([P, qg], F32, name="y",
                                                 tag="y") for _ in range(2)]
                              for kc in range(kmax + 1):
                                  ksl = slice(kc * P, (kc + 1) * P)
                                  r = kc - cpq * j  # diag band index
                                  # columns left of the diag block are fully
                                  # masked: skip them in scores, exp and
                                  # attn@V alike
                                  qo = r * P if (trim and r > 0) else 0
                                  quad = quad_pool.tile([P, 2, qg], F32,
                                                        name="quad", tag="quad")
                                  sN = 64 if "scores" in abl else qg - qo
                                  for ci in range(2):
                                      h = 2 * pi + ci
                                      hp = slice(32 * h, 32 * h + 32)
                                      nc.tensor.matmul(
                                          quad[:, ci, qo:qo + sN],
                                          kT[b][hp, ksl],
                                          qT[b][hp,
                                                j * qg + qo:j * qg + qo + sN],
                                          start=True, stop=True,
                                          tile_position=(32 * h, 0))
                                  et = exp_pool.tile([P, 2, qg], FV,
                                                     name="et", tag="et")
                                  blk = slice(r * P, (r + 1) * P)
                                  if r >= 0 and not postmask:
                                      nc.vector.tensor_tensor(
                                          quad[:, :, blk], quad[:, :, blk],
                                          cmask[:, None, :].to_broadcast(
                                              (P, 2, P)),
                                          mybir.AluOpType.add)
                                  eo = r * P if r > 0 else 0
                                  if eo > 0 and not trim:
                                      nc.gpsimd.memset(et[:, :, 0:eo], 0.0)
                                  eN = 64 if "exp" in abl else qg - eo
                                  if exp_mode == "flat" and eo == 0 and \
                                          eN == qg:
                                      nc.scalar.activation(
                                          et[:].rearrange("p a b -> p (a b)"),
                                          quad[:].rearrange(
                                              "p a b -> p (a b)"),
                                          mybir.ActivationFunctionType.Exp,
                                          scale=scale)
                                  elif exp_mode in ("2ci", "flat"):
                                      for ci in range(2):
                                          nc.scalar.activation(
                                              et[:, ci, eo:eo + eN],
                                              quad[:, ci, eo:eo + eN],
                                              mybir.ActivationFunctionType.Exp,
                                              scale=scale)
                                  else:
                                      nc.scalar.activation(
                                          et[:, :, eo:eo + eN],
                                          quad[:, :, eo:eo + eN],
                                          mybir.ActivationFunctionType.Exp,
                                          scale=scale)
                                  if r >= 0 and postmask and "mask" not in abl:
                                      # zero the strictly-upper triangle of
                                      # the diagonal 128x128 block (fp16 2x
                                      # DVE mode, post-exp)
                                      nc.vector.tensor_tensor(
                                          et[:, :, blk], et[:, :, blk],
                                          bmask[:, None, :].to_broadcast(
                                              (P, 2, P)),
                                          mybir.AluOpType.mult)
                                  st = kc == 0
                                  sp = kc == kmax
                                  aN = 64 if "av" in abl else qg - qo
                                  for ci in range(2):
                                      h = 2 * pi + ci
                                      nc.tensor.matmul(
                                          y_p[ci][:, qo:qo + aN],
                                          vP[b][:, kc, h, :],
                                          et[:, ci, qo:qo + aN],
                                          start=st, stop=sp,
                                          skip_group_check=True)
                              if j == 0:
                                  # exact fp32 recompute of query rows 0:128
                                  # (kc=0 only); overwrites the fp16 result
                                  # in y columns 0:128
                                  q32 = quad_pool.tile([P, 2, qg], F32,
                                                       name="q32", tag="quad")
                                  for ci in range(2):
                                      h = 2 * pi + ci
                                      hp = slice(32 * h, 32 * h + 32)
                                      nc.tensor.matmul(
                                          q32[:, ci, 0:P], kT32[b][hp, :],
                                          qT32[b][hp, :],
                                          start=True, stop=True,
                                          tile_position=(32 * h, 0))
                                  nc.vector.tensor_tensor(
                                      q32[:, :, 0:P], q32[:, :, 0:P],
                                      cmask[:, None, :].to_broadcast(
                                          (P, 2, P)),
                                      mybir.AluOpType.add)
                                  et32 = r_pool.tile([P, 2, P], F32,
                                                     name="et32", tag="et32")
                                  nc.scalar.activation(
                                      et32[:], q32[:, :, 0:P],
                                      mybir.ActivationFunctionType.Exp,
                                      scale=scale)
                                  for ci in range(2):
                                      h = 2 * pi + ci
                                      nc.tensor.matmul(
                                          y_p[ci][:, 0:P],
                                          vP32[b][:, h, :], et32[:, ci, :],
                                          start=True, stop=True,
                                          skip_group_check=True)
                              # ---- normalize this pair ----
                              s_t = s_p[pi]
                              nc.vector.tensor_copy(s_t[32:33, :],
                                                    y_p[0][32:33, :])
                              nc.vector.tensor_copy(s_t[64:65, :],
                                                    y_p[1][64:65, :])
                              ps_r = aux_pool.tile([P, qg], F32,
                                                   name="psr", tag="psr")
                              nc.tensor.matmul(ps_r[0:2 * DH, :],
                                               exp_e[:], s_t[:],
                                               start=True, stop=True)
                              rec = r_pool.tile([2 * DH, qg], F32,
                                                name="rec", tag="rec")
                              nc.vector.reciprocal(rec[:], ps_r[0:2 * DH, :])
                              nc.vector.tensor_mul(
                                  ysbP[pi][0:DH, :], y_p[0][0:DH, :],
                                  rec[0:DH, :])
                              nc.vector.tensor_mul(
                                  ysbP[pi][DH:2 * DH, :], y_p[1][0:DH, :],
                                  rec[DH:2 * DH, :])
                          # ---- output projection ----
                          for tch in range(qg // P):
                              t0 = j * qg + tch * P
                              csl = slice(tch * P, (tch + 1) * P)
                              po = aux_pool.tile([P, D], F32,
                                                 name="proj", tag="proj")
                              for pi in range(2):
                                  nc.tensor.matmul(
                                      po[:], ysbP[pi][:, csl], wpP[pi][:],
                                      start=(pi == 0), stop=(pi == 1),
                                      skip_group_check=True)
                              ob = out_pool.tile([P, D], F32,
                                                 name="ob", tag="ob")
                              nc.vector.tensor_copy(ob[:], po[:])
                              nc.sync.dma_start(out_d[b, t0:t0 + P, :], ob[:])
    nc.compile()
    return nc


def build_attention_v2(bpc=BPC, t=T, qg=512, loop_n=0, attn_dt="f16",
                       quad_bufs=2, exp_bufs=6,
                       trim=True, postmask=True, fp16a=True, fp16np=True,
                       xin_bufs=10, aux_bufs=2,
                       pump_in_loop=True, tr_boundary=False,
                       split_proj=False):
    """Software-pipelined variant: slab-prefetched phase A + deferred
    normalize/projection, all interleaved into the attention kc loop so the
    PE never idles waiting on ACT (exp latency hidden behind the next
    chunk's scores) and ACT never idles waiting for phase A / normalize.

    PSUM budget/partition (16KB): quad 2x4KB + y 2x2KB + aux 2x2KB.
    All phase-A / normalize / projection / recompute PSUM goes through the
    shared single-bank "aux" slots as views of a [P, 512] fp32 tile.
    """
    from collections import deque

    assert qg == 512 and t % qg == 0
    nqg = t // qg
    nkc = t // P
    cpq = qg // P
    scale = 1.0 / float(np.sqrt(DH))
    FA = {"f32": F32, "f16": mybir.dt.float16,
          "f32r": mybir.dt.float32r}[attn_dt]
    FV = FA
    FP = mybir.dt.float16 if fp16a else F32
    FN = mybir.dt.float16 if fp16np else F32

    nc = bacc.Bacc("TRN2", target_bir_lowering=False, debug=False)
    x_d = nc.dram_tensor("x", [bpc, t, D], F32, kind="ExternalInput")
    wa_d = nc.dram_tensor("w_attn", [D, 3 * D], F32, kind="ExternalInput")
    wp_d = nc.dram_tensor("w_proj", [D, D], F32, kind="ExternalInput")
    out_d = nc.dram_tensor("out", [bpc, t, D], F32, kind="ExternalOutput")

    with tile.TileContext(nc) as tc:
        with tc.tile_pool(name="resident", bufs=1) as res:
            # ---- constants (same layout tricks as v1) ----
            wa_sb = res.tile([D, 3 * D], F32, name="wa", tag="wa")
            nc.sync.dma_start(wa_sb[:], wa_d[:])
            wa16 = res.tile([D, 3 * D], FP, name="wa16", tag="wa16")
            nc.vector.tensor_copy(wa16[:], wa_sb[:])
            wpF = res.tile([D, D], F32, name="wpf", tag="wpf")
            nc.sync.dma_start(wpF[:], wp_d[:])
            wpP = [res.tile([D, D], FN, name=f"wp{pi}", tag=f"wp{pi}")
                   for pi in range(2)]
            for pi in range(2):
                nc.vector.memset(wpP[pi][:], 0.0)
                for ci in range(2):
                    h = 2 * pi + ci
                    nc.vector.tensor_copy(wpP[pi][DH * ci:DH * (ci + 1), :],
                                          wpF[h * DH:(h + 1) * DH, :])

            ident = res.tile([P, P], F32, name="ident", tag="ident")
            nc.gpsimd.memset(ident[:], 0.0)
            nc.gpsimd.affine_select(
                out=ident[:], in_=ident[:],
                compare_op=mybir.AluOpType.not_equal, fill=1.0,
                base=0, pattern=[[-1, P]], channel_multiplier=1)

            cmask = res.tile([P, P], F32, name="cmask", tag="cmask")
            nc.gpsimd.memset(cmask[:], 0.0)
            nc.gpsimd.affine_select(
                out=cmask[:], in_=cmask[:],
                compare_op=mybir.AluOpType.is_ge, fill=NEG,
                base=0, pattern=[[1, P]], channel_multiplier=-1)
            bmask = res.tile([P, P], FV, name="bmask", tag="bmask")
            nc.gpsimd.memset(bmask[:], 1.0)
            nc.gpsimd.affine_select(
                out=bmask[:], in_=bmask[:],
                compare_op=mybir.AluOpType.is_ge, fill=0.0,
                base=0, pattern=[[1, P]], channel_multiplier=-1)

            exp_e = res.tile([P, 2 * DH], FN, name="exp_e", tag="exp_e")
            nc.gpsimd.memset(exp_e[:], 0.0)
            nc.gpsimd.memset(exp_e[32:33, 0:DH], 1.0)
            nc.gpsimd.memset(exp_e[64:65, DH:2 * DH], 1.0)

            # double-buffered (by group parity) sums + unnormalized-y staging
            s_p = [[res.tile([P, qg], FN, name=f"s_p{pi}_{par}",
                             tag=f"s_p{pi}_{par}") for par in range(2)]
                   for pi in range(2)]
            ysbU = [[res.tile([2 * DH, qg], FN, name=f"ysbu{pi}_{par}",
                              tag=f"ysbu{pi}_{par}") for par in range(2)]
                    for pi in range(2)]
            ysbP = [res.tile([P, qg], FN, name=f"ysbp{pi}", tag=f"ysbp{pi}")
                    for pi in range(2)]
            for pi in range(2):
                for par in range(2):
                    nc.vector.memset(s_p[pi][par][:], 0.0)
                nc.vector.memset(ysbP[pi][:], 0.0)

            qT32 = [res.tile([P, P], F32, name=f"qT32_{b}", tag=f"qT32_{b}")
                    for b in range(bpc)]
            kT32 = [res.tile([P, P], F32, name=f"kT32_{b}", tag=f"kT32_{b}")
                    for b in range(bpc)]
            vP32 = [res.tile([P, H, P], F32, name=f"vP32_{b}", tag=f"vP32_{b}")
                    for b in range(bpc)]
            for b in range(bpc):
                nc.gpsimd.memset(vP32[b][:], 0.0)
                nc.gpsimd.memset(vP32[b][:, 0::2, 32:33], 1.0)
                nc.gpsimd.memset(vP32[b][:, 1::2, 64:65], 1.0)

            xT = [res.tile([P, t], FP, name=f"xT{b}", tag=f"xT{b}")
                  for b in range(bpc)]
            qT = [res.tile([P, t], FA, name=f"qT{b}", tag=f"qT{b}")
                  for b in range(bpc)]
            kT = [res.tile([P, t], FA, name=f"kT{b}", tag=f"kT{b}")
                  for b in range(bpc)]
            vP = [res.tile([P, nkc, H, P], FV, name=f"vp{b}", tag=f"vp{b}")
                  for b in range(bpc)]
            for b in range(bpc):
                nc.gpsimd.memset(vP[b][:], 0.0)
                nc.gpsimd.memset(vP[b][:, :, 0::2, 32:33], 1.0)
                nc.gpsimd.memset(vP[b][:, :, 1::2, 64:65], 1.0)

            import contextlib
            loop_cm = (tc.For_i(0, loop_n, 1) if loop_n
                       else contextlib.nullcontext())
            with loop_cm, (
                tc.tile_pool(name="quad", bufs=quad_bufs, space="PSUM")
            ) as quad_pool, (
                tc.tile_pool(name="ypsum", bufs=2, space="PSUM")
            ) as y_pool, (
                tc.tile_pool(name="aux", bufs=aux_bufs, space="PSUM")
            ) as aux_pool, (
                tc.tile_pool(name="xin", bufs=xin_bufs)
            ) as xin_pool, (
                tc.tile_pool(name="expt", bufs=exp_bufs)
            ) as exp_pool, (
                tc.tile_pool(name="outsb", bufs=6)
            ) as out_pool, (
                tc.tile_pool(name="rsb", bufs=4)
            ) as r_pool:

                def aux():
                    return aux_pool.tile([P, 512], F32, name="aux", tag="aux")

                pend_slab = deque()
                pend_other = deque()

                def pump(n=1):
                    # one slab item + one deferred item per slot: keeps both
                    # queues draining so the deferral window stays bounded
                    for _ in range(n):
                        if pend_slab:
                            pend_slab.popleft()()
                        if pend_other:
                            pend_other.popleft()()

                def flush_slab():
                    while pend_slab:
                        pend_slab.popleft()()

                def flush_all():
                    flush_slab()
                    while pend_other:
                        pend_other.popleft()()

                def push_slab(b2, j2):
                    """Prefetch-DMA + queue the phase-A work for one
                    512-token slab: transposes, Q/K projection, V chunks."""
                    sl0 = j2 * qg
                    xins = []
                    for i in range(cpq):
                        xi = xin_pool.tile([P, D], F32, name="xin", tag="xin")
                        nc.sync.dma_start(
                            xi[:], x_d[b2, sl0 + i * P:sl0 + (i + 1) * P, :])
                        xins.append(xi)

                    def mk_tr(i, xi):
                        def go():
                            a = aux()
                            nc.tensor.transpose(a[:, 0:P], xi[:], ident[:])
                            nc.vector.tensor_copy(
                                xT[b2][:, sl0 + i * P:sl0 + (i + 1) * P],
                                a[:, 0:P])
                        return go
                    if tr_boundary:
                        # transposes only at group boundaries (outside open
                        # PSUM accumulation groups)
                        for i, xi in enumerate(xins):
                            mk_tr(i, xi)()
                    else:
                        for i, xi in enumerate(xins):
                            pend_slab.append(mk_tr(i, xi))

                    def mk_qk(wofs, dst, dst32):
                        def go():
                            a = aux()
                            nc.tensor.matmul(
                                a[:], wa16[:, wofs:wofs + D],
                                xT[b2][:, sl0:sl0 + qg],
                                start=True, stop=True)
                            nc.vector.tensor_copy(dst[:, sl0:sl0 + qg], a[:])
                            if j2 == 0:
                                nc.vector.tensor_copy(dst32[:], a[:, 0:P])
                        return go
                    pend_slab.append(mk_qk(0, qT[b2], qT32[b2]))
                    pend_slab.append(mk_qk(D, kT[b2], kT32[b2]))

                    def mk_v(i):
                        def go():
                            kc2 = j2 * cpq + i
                            a = aux()
                            nc.tensor.matmul(
                                a[:, 0:P], xT[b2][:, kc2 * P:(kc2 + 1) * P],
                                wa16[:, 2 * D:3 * D], start=True, stop=True)
                            nc.vector.tensor_copy(
                                vP[b2][:, kc2, :, 0:DH],
                                a[:, 0:P].rearrange("p (h d) -> p h d", h=H))
                            if kc2 == 0:
                                nc.vector.tensor_copy(
                                    vP32[b2][:, :, 0:DH],
                                    a[:, 0:P].rearrange("p (h d) -> p h d",
                                                       h=H))
                        return go
                    for i in range(cpq):
                        pend_slab.append(mk_v(i))

                def push_pair_tail(b, j, pi, y_p):
                    """Emit the DVE-only y/sums staging now (frees the y PSUM
                    banks), defer recompute/normalize/projection stages."""
                    # the previous pair's deferred stages must complete before
                    # this pair's staging reuses the parity tiles two pairs
                    # from now: bound the window to one pair
                    while pend_other:
                        pend_other.popleft()()
                    par = j & 1
                    st = s_p[pi][par]
                    yu = ysbU[pi][par]
                    lo = P if j == 0 else 0
                    nc.vector.tensor_copy(st[32:33, lo:], y_p[0][32:33, lo:])
                    nc.vector.tensor_copy(st[64:65, lo:], y_p[1][64:65, lo:])
                    nc.vector.tensor_copy(yu[0:DH, lo:], y_p[0][0:DH, lo:])
                    nc.vector.tensor_copy(yu[DH:2 * DH, lo:],
                                          y_p[1][0:DH, lo:])
                    stash = {}
                    if j == 0:
                        # deferred exact-fp32 recompute of query rows 0:128
                        def r1():
                            a = aux()
                            q32 = a.rearrange("p (c q) -> p c q",
                                              c=2)[:, :, 0:P]
                            for ci in range(2):
                                h = 2 * pi + ci
                                hp = slice(32 * h, 32 * h + 32)
                                nc.tensor.matmul(
                                    q32[:, ci, :], kT32[b][hp, :],
                                    qT32[b][hp, :], start=True, stop=True,
                                    tile_position=(32 * h, 0))
                            stash["q32"] = q32

                        def r2():
                            q32 = stash.pop("q32")
                            nc.vector.tensor_tensor(
                                q32[:], q32[:],
                                cmask[:, None, :].to_broadcast((P, 2, P)),
                                mybir.AluOpType.add)
                            et32 = r_pool.tile([P, 2, P], F32, name="et32",
                                               tag="et32")
                            nc.scalar.activation(
                                et32[:], q32[:],
                                mybir.ActivationFunctionType.Exp, scale=scale)
                            stash["et32"] = et32

                        def r3():
                            et32 = stash.pop("et32")
                            a = aux()
                            yR = a.rearrange("p (c q) -> p c q",
                                             c=2)[:, :, 0:P]
                            for ci in range(2):
                                h = 2 * pi + ci
                                nc.tensor.matmul(
                                    yR[:, ci, :], vP32[b][:, h, :],
                                    et32[:, ci, :], start=True, stop=True,
                                    skip_group_check=True)
                            nc.vector.tensor_copy(st[32:33, 0:P],
                                                  yR[32:33, 0, :])
                            nc.vector.tensor_copy(st[64:65, 0:P],
                                                  yR[64:65, 1, :])
                            nc.vector.tensor_copy(yu[0:DH, 0:P],
                                                  yR[0:DH, 0, :])
                            nc.vector.tensor_copy(yu[DH:2 * DH, 0:P],
                                                  yR[0:DH, 1, :])
                        pend_other.extend([r1, r2, r3])

                    def n1():
                        a = aux()
                        nc.tensor.matmul(a[0:2 * DH, :], exp_e[:], st[:],
                                         start=True, stop=True)
                        stash["psr"] = a

                    def n2():
                        a = stash.pop("psr")
                        rec = r_pool.tile([2 * DH, qg], F32, name="rec",
                                          tag="rec")
                        nc.vector.reciprocal(rec[:], a[0:2 * DH, :])
                        nc.vector.tensor_mul(ysbP[pi][0:DH, :], yu[0:DH, :],
                                             rec[0:DH, :])
                        nc.vector.tensor_mul(ysbP[pi][DH:2 * DH, :],
                                             yu[DH:2 * DH, :],
                                             rec[DH:2 * DH, :])
                    pend_other.extend([n1, n2])

                    if pi == 1:
                        def mk_proj(tch):
                            def go():
                                t0 = j * qg + tch * P
                                csl = slice(tch * P, (tch + 1) * P)
                                a = aux()
                                ob = out_pool.tile([P, D], F32, name="ob",
                                                   tag="ob")
                                if split_proj:
                                    # two single-shot matmuls into disjoint
                                    # halves + DVE add (avoids a nested PSUM
                                    # accumulation group)
                                    for pp in range(2):
                                        nc.tensor.matmul(
                                            a[:, pp * D:(pp + 1) * D],
                                            ysbP[pp][:, csl], wpP[pp][:],
                                            start=True, stop=True,
                                            skip_group_check=True)
                                    nc.vector.tensor_tensor(
                                        ob[:], a[:, 0:D], a[:, D:2 * D],
                                        mybir.AluOpType.add)
                                else:
                                    for pp in range(2):
                                        nc.tensor.matmul(
                                            a[:, 0:D], ysbP[pp][:, csl],
                                            wpP[pp][:],
                                            start=(pp == 0), stop=(pp == 1),
                                            skip_group_check=True)
                                    nc.vector.tensor_copy(ob[:], a[:, 0:D])
                                nc.sync.dma_start(out_d[b, t0:t0 + P, :],
                                                  ob[:])
                            return go
                        for tch in range(cpq):
                            pend_other.append(mk_proj(tch))

                # ---------------- main pipeline ----------------
                push_slab(0, 0)
                flush_slab()
                ngroups = bpc * nqg
                for g in range(ngroups):
                    b, j = divmod(g, nqg)
                    if g + 1 < ngroups:
                        push_slab(*divmod(g + 1, nqg))
                    kmax = cpq * (j + 1) - 1
                    for pi in range(2):
                        y_p = [y_pool.tile([P, qg], F32, name="y", tag="y")
                               for _ in range(2)]

                        def emit_av(kc, et, qo):
                            st_f = kc == 0
                            sp_f = kc == kmax
                            for ci in range(2):
                                h = 2 * pi + ci
                                nc.tensor.matmul(
                                    y_p[ci][:, qo:], vP[b][:, kc, h, :],
                                    et[:, ci, qo:], start=st_f, stop=sp_f,
                                    skip_group_check=True)

                        prev = None
                        for kc in range(kmax + 1):
                            ksl = slice(kc * P, (kc + 1) * P)
                            r = kc - cpq * j
                            qo = r * P if (trim and r > 0) else 0
                            quad = quad_pool.tile([P, 2, qg], F32,
                                                  name="quad", tag="quad")
                            for ci in range(2):
                                h = 2 * pi + ci
                                hp = slice(32 * h, 32 * h + 32)
                                nc.tensor.matmul(
                                    quad[:, ci, qo:], kT[b][hp, ksl],
                                    qT[b][hp, j * qg + qo:(j + 1) * qg],
                                    start=True, stop=True,
                                    tile_position=(32 * h, 0))
                            et = exp_pool.tile([P, 2, qg], FV, name="et",
                                               tag="et")
                            blk = slice(r * P, (r + 1) * P)
                            if r >= 0 and not postmask:
                                nc.vector.tensor_tensor(
                                    quad[:, :, blk], quad[:, :, blk],
                                    cmask[:, None, :].to_broadcast((P, 2, P)),
                                    mybir.AluOpType.add)
                            eo = r * P if r > 0 else 0
                            if eo > 0 and not trim:
                                nc.gpsimd.memset(et[:, :, 0:eo], 0.0)
                            nc.scalar.activation(
                                et[:, :, eo:], quad[:, :, eo:],
                                mybir.ActivationFunctionType.Exp, scale=scale)
                            if r >= 0 and postmask:
                                nc.vector.tensor_tensor(
                                    et[:, :, blk], et[:, :, blk],
                                    bmask[:, None, :].to_broadcast((P, 2, P)),
                                    mybir.AluOpType.mult)
                            if prev is not None:
                                emit_av(*prev)
                                if pump_in_loop:
                                    pump(1)
                            prev = (kc, et, qo)
                        emit_av(*prev)
                        if pump_in_loop:
                            pump(1)
                        push_pair_tail(b, j, pi, y_p)
                    flush_slab()
                flush_all()
    nc.compile()
    return nc


def build_attention_v3(bpc=BPC, t=T, qg=512, loop_n=0,
                       quad_bufs=2, exp_bufs=6, y_bufs=2, aux_bufs=2,
                       xin_bufs=10, pump_mm=True, rec32=True,
                       pump_slab=1, pump_other=1):
    """v3: v1's compute structure, fully software-pipelined.

    - attn@V for chunk kc is emitted AFTER scores/exp of chunk kc+1, so the
      PE FIFO never head-of-line blocks the ACT engine (exp is the roofline:
      ~139k elem-cycles/partition per core).
    - normalize / out-projection / next slab's phase A are deferred into the
      kc loop (pump), using ONLY single-shot PSUM matmuls in the shared aux
      pool (the merged out projection is one K=128 matmul against a combined
      4-head ysb tile, so no accumulation group ever lands in a shared bank
      -- the pattern that broke the v2 builder in walrus).
    - phase A per 512-token slab is prefetched during the previous group.

    PSUM: quad 2x2 + y 2x1 + aux 2x1 = 8 banks.
    """
    from collections import deque

    assert qg == 512 and t % qg == 0
    nqg = t // qg
    nkc = t // P
    cpq = qg // P
    scale = 1.0 / float(np.sqrt(DH))
    F16 = mybir.dt.float16
    FA = F16   # attention operand dtype
    FV = F16   # exp tiles + V'
    FP = F16   # phase A
    FN = F16   # normalize / projection

    nc = bacc.Bacc("TRN2", target_bir_lowering=False, debug=False)
    x_d = nc.dram_tensor("x", [bpc, t, D], F32, kind="ExternalInput")
    wa_d = nc.dram_tensor("w_attn", [D, 3 * D], F32, kind="ExternalInput")
    wp_d = nc.dram_tensor("w_proj", [D, D], F32, kind="ExternalInput")
    out_d = nc.dram_tensor("out", [bpc, t, D], F32, kind="ExternalOutput")

    with tile.TileContext(nc) as tc:
        with tc.tile_pool(name="resident", bufs=1) as res:
            # ---- constants ----
            wa_sb = res.tile([D, 3 * D], F32, name="wa", tag="wa")
            nc.sync.dma_start(wa_sb[:], wa_d[:])
            wa16 = res.tile([D, 3 * D], FP, name="wa16", tag="wa16")
            nc.vector.tensor_copy(wa16[:], wa_sb[:])
            # merged projection weight: plain w_proj rows (head h contributes
            # rows 32h:32h+32, matching ysbA's row layout below)
            wpF = res.tile([D, D], F32, name="wpf", tag="wpf")
            nc.sync.dma_start(wpF[:], wp_d[:])
            wp16 = res.tile([D, D], FN, name="wp16", tag="wp16")
            nc.vector.tensor_copy(wp16[:], wpF[:])

            ident = res.tile([P, P], F32, name="ident", tag="ident")
            nc.gpsimd.memset(ident[:], 0.0)
            nc.gpsimd.affine_select(
                out=ident[:], in_=ident[:],
                compare_op=mybir.AluOpType.not_equal, fill=1.0,
                base=0, pattern=[[-1, P]], channel_multiplier=1)

            cmask = res.tile([P, P], F32, name="cmask", tag="cmask")
            nc.gpsimd.memset(cmask[:], 0.0)
            nc.gpsimd.affine_select(
                out=cmask[:], in_=cmask[:],
                compare_op=mybir.AluOpType.is_ge, fill=NEG,
                base=0, pattern=[[1, P]], channel_multiplier=-1)
            bmask = res.tile([P, P], FV, name="bmask", tag="bmask")
            nc.gpsimd.memset(bmask[:], 1.0)
            nc.gpsimd.affine_select(
                out=bmask[:], in_=bmask[:],
                compare_op=mybir.AluOpType.is_ge, fill=0.0,
                base=0, pattern=[[1, P]], channel_multiplier=-1)

            exp_e = res.tile([P, 2 * DH], FN, name="exp_e", tag="exp_e")
            nc.gpsimd.memset(exp_e[:], 0.0)
            nc.gpsimd.memset(exp_e[32:33, 0:DH], 1.0)
            nc.gpsimd.memset(exp_e[64:65, DH:2 * DH], 1.0)

            # parity-double-buffered sums + unnormalized-y staging
            s_p = [[res.tile([P, qg], FN, name=f"s_p{pi}_{par}",
                             tag=f"s_p{pi}_{par}") for par in range(2)]
                   for pi in range(2)]
            ysbU = [[res.tile([2 * DH, qg], FN, name=f"ysbu{pi}_{par}",
                              tag=f"ysbu{pi}_{par}") for par in range(2)]
                    for pi in range(2)]
            for pi in range(2):
                for par in range(2):
                    nc.vector.memset(s_p[pi][par][:], 0.0)
            # combined normalized y: head h lives on rows 32h:32h+32
            ysbA = res.tile([P, qg], FN, name="ysba", tag="ysba")

            qT32 = [res.tile([P, P], F32, name=f"qT32_{b}", tag=f"qT32_{b}")
                    for b in range(bpc)]
            kT32 = [res.tile([P, P], F32, name=f"kT32_{b}", tag=f"kT32_{b}")
                    for b in range(bpc)]
            vP32 = [res.tile([P, H, P], F32, name=f"vP32_{b}",
                             tag=f"vP32_{b}") for b in range(bpc)]
            for b in range(bpc):
                nc.gpsimd.memset(vP32[b][:], 0.0)
                nc.gpsimd.memset(vP32[b][:, 0::2, 32:33], 1.0)
                nc.gpsimd.memset(vP32[b][:, 1::2, 64:65], 1.0)

            xT = [res.tile([P, t], FP, name=f"xT{b}", tag=f"xT{b}")
                  for b in range(bpc)]
            qT = [res.tile([P, t], FA, name=f"qT{b}", tag=f"qT{b}")
                  for b in range(bpc)]
            kT = [res.tile([P, t], FA, name=f"kT{b}", tag=f"kT{b}")
                  for b in range(bpc)]
            vP = [res.tile([P, nkc, H, P], FV, name=f"vp{b}", tag=f"vp{b}")
                  for b in range(bpc)]
            for b in range(bpc):
                nc.gpsimd.memset(vP[b][:], 0.0)
                nc.gpsimd.memset(vP[b][:, :, 0::2, 32:33], 1.0)
                nc.gpsimd.memset(vP[b][:, :, 1::2, 64:65], 1.0)

            import contextlib
            loop_cm = (tc.For_i(0, loop_n, 1) if loop_n
                       else contextlib.nullcontext())
            with loop_cm, (
                tc.tile_pool(name="quad", bufs=quad_bufs, space="PSUM")
            ) as quad_pool, (
                tc.tile_pool(name="ypsum", bufs=y_bufs, space="PSUM")
            ) as y_pool, (
                tc.tile_pool(name="aux", bufs=aux_bufs, space="PSUM")
            ) as aux_pool, (
                tc.tile_pool(name="xin", bufs=xin_bufs)
            ) as xin_pool, (
                tc.tile_pool(name="expt", bufs=exp_bufs)
            ) as exp_pool, (
                tc.tile_pool(name="outsb", bufs=6)
            ) as out_pool, (
                tc.tile_pool(name="rsb", bufs=4)
            ) as r_pool:

                def aux():
                    return aux_pool.tile([P, 512], F32, name="aux", tag="aux")

                # (has_mm, fn) queues.  pend_slab: next slab's phase A.
                # pend_other: deferred normalize / recompute / projection.
                pend_slab = deque()
                pend_other = deque()

                def pop_ok(q):
                    return q and (pump_mm or not q[0][0])

                def pump():
                    for _ in range(pump_slab):
                        if pop_ok(pend_slab):
                            pend_slab.popleft()[1]()
                    for _ in range(pump_other):
                        if pop_ok(pend_other):
                            pend_other.popleft()[1]()

                def flush_slab():
                    while pend_slab:
                        pend_slab.popleft()[1]()

                def flush_other():
                    while pend_other:
                        pend_other.popleft()[1]()

                def push_slab(b2, j2):
                    sl0 = j2 * qg
                    xins = []
                    for i in range(cpq):
                        xi = xin_pool.tile([P, D], F32, name="xin", tag="xin")
                        nc.sync.dma_start(
                            xi[:], x_d[b2, sl0 + i * P:sl0 + (i + 1) * P, :])
                        xins.append(xi)

                    def mk_tr(i, xi):
                        def go():
                            a = aux()
                            nc.tensor.transpose(a[:, 0:P], xi[:], ident[:])
                            nc.vector.tensor_copy(
                                xT[b2][:, sl0 + i * P:sl0 + (i + 1) * P],
                                a[:, 0:P])
                        return go
                    for i, xi in enumerate(xins):
                        pend_slab.append((True, mk_tr(i, xi)))

                    def mk_qk(wofs, dst, dst32):
                        def go():
                            a = aux()
                            nc.tensor.matmul(
                                a[:], wa16[:, wofs:wofs + D],
                                xT[b2][:, sl0:sl0 + qg],
                                start=True, stop=True)
                            nc.vector.tensor_copy(dst[:, sl0:sl0 + qg], a[:])
                            if j2 == 0 and rec32:
                                nc.vector.tensor_copy(dst32[:], a[:, 0:P])
                        return go
                    pend_slab.append((True, mk_qk(0, qT[b2], qT32[b2])))
                    pend_slab.append((True, mk_qk(D, kT[b2], kT32[b2])))

                    def mk_v(i):
                        def go():
                            kc2 = j2 * cpq + i
                            a = aux()
                            nc.tensor.matmul(
                                a[:, 0:P], xT[b2][:, kc2 * P:(kc2 + 1) * P],
                                wa16[:, 2 * D:3 * D], start=True, stop=True)
                            nc.vector.tensor_copy(
                                vP[b2][:, kc2, :, 0:DH],
                                a[:, 0:P].rearrange("p (h d) -> p h d", h=H))
                            if kc2 == 0 and rec32:
                                nc.vector.tensor_copy(
                                    vP32[b2][:, :, 0:DH],
                                    a[:, 0:P].rearrange("p (h d) -> p h d",
                                                        h=H))
                        return go
                    for i in range(cpq):
                        pend_slab.append((True, mk_v(i)))

                def pair_tail(b, j, pi, y_p):
                    # older deferred stages must be done before this pair's
                    # staging overwrites the OTHER parity's consumers' inputs
                    # two groups from now; flushing here bounds the window.
                    flush_other()
                    par = j & 1
                    st = s_p[pi][par]
                    yu = ysbU[pi][par]
                    lo = P if (j == 0 and rec32) else 0
                    # staging: frees the two y PSUM banks
                    nc.vector.tensor_copy(st[32:33, lo:], y_p[0][32:33, lo:])
                    nc.vector.tensor_copy(yu[0:DH, lo:], y_p[0][0:DH, lo:])
                    nc.vector.tensor_copy(st[64:65, lo:], y_p[1][64:65, lo:])
                    nc.vector.tensor_copy(yu[DH:2 * DH, lo:],
                                          y_p[1][0:DH, lo:])
                    stash = {}
                    if j == 0 and rec32:
                        # deferred exact-fp32 recompute of query rows 0:128
                        def r1():
                            a = aux()
                            for ci in range(2):
                                h = 2 * pi + ci
                                hp = slice(32 * h, 32 * h + 32)
                                nc.tensor.matmul(
                                    a[:, ci * P:(ci + 1) * P], kT32[b][hp, :],
                                    qT32[b][hp, :], start=True, stop=True,
                                    tile_position=(32 * h, 0))
                            stash["q32"] = a

                        def r2():
                            a = stash.pop("q32")
                            for ci in range(2):
                                nc.vector.tensor_tensor(
                                    a[:, ci * P:(ci + 1) * P],
                                    a[:, ci * P:(ci + 1) * P],
                                    cmask[:], mybir.AluOpType.add)
                            et32 = r_pool.tile([P, 2 * P], F32, name="et32",
                                               tag="et32")
                            nc.scalar.activation(
                                et32[:], a[:, 0:2 * P],
                                mybir.ActivationFunctionType.Exp, scale=scale)
                            stash["et32"] = et32

                        def r3():
                            et32 = stash.pop("et32")
                            a = aux()
                            for ci in range(2):
                                h = 2 * pi + ci
                                nc.tensor.matmul(
                                    a[:, ci * P:(ci + 1) * P],
                                    vP32[b][:, h, :],
                                    et32[:, ci * P:(ci + 1) * P],
                                    start=True, stop=True,
                                    skip_group_check=True)
                            nc.vector.tensor_copy(st[32:33, 0:P],
                                                  a[32:33, 0:P])
                            nc.vector.tensor_copy(yu[0:DH, 0:P],
                                                  a[0:DH, 0:P])
                            nc.vector.tensor_copy(st[64:65, 0:P],
                                                  a[64:65, P:2 * P])
                            nc.vector.tensor_copy(yu[DH:2 * DH, 0:P],
                                                  a[0:DH, P:2 * P])
                        pend_other.extend([(True, r1), (False, r2),
                                           (True, r3)])

                    def n1():
                        a = aux()
                        nc.tensor.matmul(a[0:2 * DH, :], exp_e[:], st[:],
                                         start=True, stop=True)
                        stash["psr"] = a

                    def n2():
                        a = stash.pop("psr")
                        rec = r_pool.tile([2 * DH, qg], F32, name="rec",
                                          tag="rec")
                        nc.vector.reciprocal(rec[:], a[0:2 * DH, :])
                        r0 = 64 * pi
                        nc.vector.tensor_mul(ysbA[r0:r0 + DH, :],
                                             yu[0:DH, :], rec[0:DH, :])
                        nc.vector.tensor_mul(ysbA[r0 + DH:r0 + 2 * DH, :],
                                             yu[DH:2 * DH, :],
                                             rec[DH:2 * DH, :])
                    pend_other.extend([(True, n1), (False, n2)])

                    if pi == 1:
                        def mk_proj(tch):
                            def go():
                                t0 = j * qg + tch * P
                                csl = slice(tch * P, (tch + 1) * P)
                                a = aux()
                                nc.tensor.matmul(
                                    a[:, 0:D], ysbA[:, csl], wp16[:],
                                    start=True, stop=True,
                                    skip_group_check=True)
                                ob = out_pool.tile([P, D], F32, name="ob",
                                                   tag="ob")
                                nc.vector.tensor_copy(ob[:], a[:, 0:D])
                                nc.sync.dma_start(out_d[b, t0:t0 + P, :],
                                                  ob[:])
                            return go
                        for tch in range(cpq):
                            pend_other.append((True, mk_proj(tch)))

                # ---------------- main pipeline ----------------
                push_slab(0, 0)
                flush_slab()
                ngroups = bpc * nqg
                for g in range(ngroups):
                    b, j = divmod(g, nqg)
                    if g + 1 < ngroups:
                        push_slab(*divmod(g + 1, nqg))
                    kmax = cpq * (j + 1) - 1
                    for pi in range(2):
                        y_p = [y_pool.tile([P, qg], F32, name="y", tag="y")
                               for _ in range(2)]

                        def emit_av(kc, et, qo):
                            st_f = kc == 0
                            sp_f = kc == kmax
                            for ci in range(2):
                                h = 2 * pi + ci
                                nc.tensor.matmul(
                                    y_p[ci][:, qo:], vP[b][:, kc, h, :],
                                    et[:, ci, qo:], start=st_f, stop=sp_f,
                                    skip_group_check=True)

                        prev = None
                        for kc in range(kmax + 1):
                            ksl = slice(kc * P, (kc + 1) * P)
                            r = kc - cpq * j
                            qo = r * P if r > 0 else 0
                            quad = quad_pool.tile([P, 2, qg], F32,
                                                  name="quad", tag="quad")
                            for ci in range(2):
                                h = 2 * pi + ci
                                hp = slice(32 * h, 32 * h + 32)
                                nc.tensor.matmul(
                                    quad[:, ci, qo:], kT[b][hp, ksl],
                                    qT[b][hp, j * qg + qo:(j + 1) * qg],
                                    start=True, stop=True,
                                    tile_position=(32 * h, 0))
                            et = exp_pool.tile([P, 2, qg], FV, name="et",
                                               tag="et")
                            eo = qo
                            nc.scalar.activation(
                                et[:, :, eo:], quad[:, :, eo:],
                                mybir.ActivationFunctionType.Exp, scale=scale)
                            if r >= 0:
                                blk = slice(r * P, (r + 1) * P)
                                nc.vector.tensor_tensor(
                                    et[:, :, blk], et[:, :, blk],
                                    bmask[:, None, :].to_broadcast((P, 2, P)),
                                    mybir.AluOpType.mult)
                            if prev is not None:
                                emit_av(*prev)
                                pump()
                            prev = (kc, et, qo)
                        emit_av(*prev)
                        pump()
                        pair_tail(b, j, pi, y_p)
                    flush_slab()
                flush_other()
    nc.compile()
    return nc


def build_attention_v4(bpc=BPC, t=T, qg=512, loop_n=0,
                       quad_dt="f16", quad_bufs=2, exp_bufs=6, y_bufs=3,
                       aux_bufs=3, xin_bufs=10,
                       defer_tail=True, defer_slab=True, pump_n=2,
                       exp_mode="3d", host_xt=False):
    """v4: ablatable pipeline around v1's proven compute structure.

    - av(kc-1) emitted after scores/exp(kc): PE FIFO never blocks ACT.
    - quad_dt="f16": score quads are fp16 in PSUM (1 bank instead of 2),
      freeing banks for deeper aux/quad/y buffering.
    - defer_tail: normalize + merged 1-matmul projection trickled into the
      next pair's kc loop as fine-grained items (each PE matmul and its DVE
      evacuation are separate queue items so a backed-up DVE FIFO cannot
      head-of-line block the PE).
    - defer_slab: next 512-token slab's phase A trickled the same way.
    """
    from collections import deque

    assert qg == 512 and t % qg == 0
    nqg = t // qg
    nkc = t // P
    cpq = qg // P
    scale = 1.0 / float(np.sqrt(DH))
    F16 = mybir.dt.float16
    FQ = F16 if quad_dt == "f16" else F32
    qbanks = 1 if quad_dt == "f16" else 2
    assert qbanks * quad_bufs + y_bufs + aux_bufs <= 8

    nc = bacc.Bacc("TRN2", target_bir_lowering=False, debug=False)
    if host_xt:
        # x arrives pre-transposed to [bpc, D, t] and pre-cast to fp16 on
        # the host (input marshalling inside kernel(); halves DMA bytes and
        # removes all on-device transposes + their DVE evacuations)
        x_d = nc.dram_tensor("x", [bpc, D, t], F16, kind="ExternalInput")
    else:
        x_d = nc.dram_tensor("x", [bpc, t, D], F32, kind="ExternalInput")
    wa_d = nc.dram_tensor("w_attn", [D, 3 * D], F32, kind="ExternalInput")
    wp_d = nc.dram_tensor("w_proj", [D, D], F32, kind="ExternalInput")
    out_d = nc.dram_tensor("out", [bpc, t, D], F32, kind="ExternalOutput")

    with tile.TileContext(nc) as tc:
        with tc.tile_pool(name="resident", bufs=1) as res:
            wa_sb = res.tile([D, 3 * D], F32, name="wa", tag="wa")
            nc.sync.dma_start(wa_sb[:], wa_d[:])
            wa16 = res.tile([D, 3 * D], F16, name="wa16", tag="wa16")
            nc.vector.tensor_copy(wa16[:], wa_sb[:])
            wpF = res.tile([D, D], F32, name="wpf", tag="wpf")
            nc.sync.dma_start(wpF[:], wp_d[:])
            wp16 = res.tile([D, D], F16, name="wp16", tag="wp16")
            nc.vector.tensor_copy(wp16[:], wpF[:])

            ident = res.tile([P, P], F32, name="ident", tag="ident")
            nc.gpsimd.memset(ident[:], 0.0)
            nc.gpsimd.affine_select(
                out=ident[:], in_=ident[:],
                compare_op=mybir.AluOpType.not_equal, fill=1.0,
                base=0, pattern=[[-1, P]], channel_multiplier=1)

            bmask = res.tile([P, P], F16, name="bmask", tag="bmask")
            nc.gpsimd.memset(bmask[:], 1.0)
            nc.gpsimd.affine_select(
                out=bmask[:], in_=bmask[:],
                compare_op=mybir.AluOpType.is_ge, fill=0.0,
                base=0, pattern=[[1, P]], channel_multiplier=-1)

            exp_e = res.tile([P, 2 * DH], F16, name="exp_e", tag="exp_e")
            nc.gpsimd.memset(exp_e[:], 0.0)
            nc.gpsimd.memset(exp_e[32:33, 0:DH], 1.0)
            nc.gpsimd.memset(exp_e[64:65, DH:2 * DH], 1.0)

            s_p = [[res.tile([P, qg], F16, name=f"s_p{pi}_{par}",
                             tag=f"s_p{pi}_{par}") for par in range(2)]
                   for pi in range(2)]
            ysbU = [[res.tile([2 * DH, qg], F16, name=f"ysbu{pi}_{par}",
                              tag=f"ysbu{pi}_{par}") for par in range(2)]
                    for pi in range(2)]
            for pi in range(2):
                for par in range(2):
                    nc.vector.memset(s_p[pi][par][:], 0.0)
            ysbA = res.tile([P, qg], F16, name="ysba", tag="ysba")

            xT = [res.tile([P, t], F16, name=f"xT{b}", tag=f"xT{b}")
                  for b in range(bpc)]
            qT = [res.tile([P, t], F16, name=f"qT{b}", tag=f"qT{b}")
                  for b in range(bpc)]
            kT = [res.tile([P, t], F16, name=f"kT{b}", tag=f"kT{b}")
                  for b in range(bpc)]
            vP = [res.tile([P, nkc, H, P], F16, name=f"vp{b}", tag=f"vp{b}")
                  for b in range(bpc)]
            for b in range(bpc):
                nc.gpsimd.memset(vP[b][:], 0.0)
                nc.gpsimd.memset(vP[b][:, :, 0::2, 32:33], 1.0)
                nc.gpsimd.memset(vP[b][:, :, 1::2, 64:65], 1.0)

            import contextlib
            loop_cm = (tc.For_i(0, loop_n, 1) if loop_n
                       else contextlib.nullcontext())
            with loop_cm, (
                tc.tile_pool(name="quad", bufs=quad_bufs, space="PSUM")
            ) as quad_pool, (
                tc.tile_pool(name="ypsum", bufs=y_bufs, space="PSUM")
            ) as y_pool, (
                tc.tile_pool(name="aux", bufs=aux_bufs, space="PSUM")
            ) as aux_pool, (
                tc.tile_pool(name="xin", bufs=xin_bufs)
            ) as xin_pool, (
                tc.tile_pool(name="expt", bufs=exp_bufs)
            ) as exp_pool, (
                tc.tile_pool(name="outsb", bufs=6)
            ) as out_pool, (
                tc.tile_pool(name="rsb", bufs=4)
            ) as r_pool:

                def aux():
                    return aux_pool.tile([P, 512], F32, name="aux", tag="aux")

                pend_slab = deque()
                pend_other = deque()

                def pump():
                    if pend_slab:
                        pend_slab.popleft()()
                    for _ in range(pump_n - 1):
                        if pend_other:
                            pend_other.popleft()()

                def flush_slab():
                    while pend_slab:
                        pend_slab.popleft()()

                def flush_other():
                    while pend_other:
                        pend_other.popleft()()

                def run_or_defer(items, q, defer):
                    if defer:
                        q.extend(items)
                    else:
                        for it in items:
                            it()

                def slab_items(b2, j2):
                    """Phase A for one 512-token slab, as fine-grained items.
                    DMAs are issued eagerly at push time."""
                    sl0 = j2 * qg
                    items = []
                    stash = {}
                    if not host_xt:
                        xins = []
                        for i in range(cpq):
                            xi = xin_pool.tile([P, D], F32, name="xin",
                                               tag="xin")
                            nc.sync.dma_start(
                                xi[:],
                                x_d[b2, sl0 + i * P:sl0 + (i + 1) * P, :])
                            xins.append(xi)

                        def mk_tr_mm(i, xi):
                            def go():
                                a = aux()
                                nc.tensor.transpose(a[:, 0:P], xi[:],
                                                    ident[:])
                                stash[("tr", i)] = a
                            return go

                        def mk_tr_cp(i):
                            def go():
                                a = stash.pop(("tr", i))
                                nc.vector.tensor_copy(
                                    xT[b2][:, sl0 + i * P:sl0 + (i + 1) * P],
                                    a[:, 0:P])
                            return go
                        for i, xi in enumerate(xins):
                            items.append(mk_tr_mm(i, xi))
                            items.append(mk_tr_cp(i))

                    def mk_qk_mm(wofs):
                        def go():
                            a = aux()
                            nc.tensor.matmul(
                                a[:], wa16[:, wofs:wofs + D],
                                xT[b2][:, sl0:sl0 + qg],
                                start=True, stop=True)
                            stash[("qk", wofs)] = a
                        return go

                    def mk_qk_cp(wofs, dst):
                        def go():
                            a = stash.pop(("qk", wofs))
                            nc.vector.tensor_copy(dst[:, sl0:sl0 + qg], a[:])
                        return go
                    items.append(mk_qk_mm(0))
                    items.append(mk_qk_cp(0, qT[b2]))
                    items.append(mk_qk_mm(D))
                    items.append(mk_qk_cp(D, kT[b2]))

                    def mk_v_mm(i):
                        def go():
                            kc2 = j2 * cpq + i
                            a = aux()
                            nc.tensor.matmul(
                                a[:, 0:P], xT[b2][:, kc2 * P:(kc2 + 1) * P],
                                wa16[:, 2 * D:3 * D], start=True, stop=True)
                            stash[("v", i)] = a
                        return go

                    def mk_v_cp(i):
                        def go():
                            kc2 = j2 * cpq + i
                            a = stash.pop(("v", i))
                            nc.vector.tensor_copy(
                                vP[b2][:, kc2, :, 0:DH],
                                a[:, 0:P].rearrange("p (h d) -> p h d", h=H))
                        return go
                    for i in range(cpq):
                        items.append(mk_v_mm(i))
                        items.append(mk_v_cp(i))
                    return items

                def tail_items(b, j, pi, y_p):
                    """Normalize + (pi==1) merged projection, fine-grained.
                    The four staging copies run immediately (free y banks).
                    Caller flushes pend_other first, so the parity tiles'
                    previous consumers are already emitted."""
                    par = j & 1
                    st = s_p[pi][par]
                    yu = ysbU[pi][par]
                    nc.vector.tensor_copy(st[32:33, :], y_p[0][32:33, :])
                    nc.vector.tensor_copy(yu[0:DH, :], y_p[0][0:DH, :])
                    nc.vector.tensor_copy(st[64:65, :], y_p[1][64:65, :])
                    nc.vector.tensor_copy(yu[DH:2 * DH, :], y_p[1][0:DH, :])
                    stash = {}
                    items = []

                    def n1():
                        a = aux()
                        nc.tensor.matmul(a[0:2 * DH, :], exp_e[:], st[:],
                                         start=True, stop=True)
                        stash["psr"] = a

                    def n2():
                        a = stash.pop("psr")
                        rec = r_pool.tile([2 * DH, qg], F32, name="rec",
                                          tag="rec")
                        nc.vector.reciprocal(rec[:], a[0:2 * DH, :])
                        r0 = 64 * pi
                        nc.vector.tensor_mul(ysbA[r0:r0 + DH, :],
                                             yu[0:DH, :], rec[0:DH, :])
                        nc.vector.tensor_mul(ysbA[r0 + DH:r0 + 2 * DH, :],
                                             yu[DH:2 * DH, :],
                                             rec[DH:2 * DH, :])
                    items.extend([n1, n2])

                    if pi == 1:
                        def mk_proj_mm(tch):
                            def go():
                                csl = slice(tch * P, (tch + 1) * P)
                                a = aux()
                                nc.tensor.matmul(
                                    a[:, 0:D], ysbA[:, csl], wp16[:],
                                    start=True, stop=True,
                                    skip_group_check=True)
                                stash[("po", tch)] = a
                            return go

                        def mk_proj_cp(tch):
                            def go():
                                t0 = j * qg + tch * P
                                a = stash.pop(("po", tch))
                                ob = out_pool.tile([P, D], F32, name="ob",
                                                   tag="ob")
                                nc.vector.tensor_copy(ob[:], a[:, 0:D])
                                nc.sync.dma_start(out_d[b, t0:t0 + P, :],
                                                  ob[:])
                            return go
                        for tch in range(cpq):
                            items.append(mk_proj_mm(tch))
                            items.append(mk_proj_cp(tch))
                    return items

                # ---------------- main pipeline ----------------
                if host_xt:
                    # one 512KB DMA per batch straight into the resident
                    # transposed activation tile (4KB contiguous per
                    # partition line -> near-peak DMA efficiency)
                    for b2 in range(bpc):
                        nc.sync.dma_start(xT[b2][:], x_d[b2])
                for it in slab_items(0, 0):
                    it()
                ngroups = bpc * nqg
                for g in range(ngroups):
                    b, j = divmod(g, nqg)
                    if g + 1 < ngroups:
                        run_or_defer(slab_items(*divmod(g + 1, nqg)),
                                     pend_slab, defer_slab)
                    kmax = cpq * (j + 1) - 1
                    for pi in range(2):
                        y_p = [y_pool.tile([P, qg], F32, name="y", tag="y")
                               for _ in range(2)]

                        def emit_av(kc, et, qo):
                            st_f = kc == 0
                            sp_f = kc == kmax
                            for ci in range(2):
                                h = 2 * pi + ci
                                nc.tensor.matmul(
                                    y_p[ci][:, qo:], vP[b][:, kc, h, :],
                                    et[:, ci, qo:], start=st_f, stop=sp_f,
                                    skip_group_check=True)

                        prev = None
                        for kc in range(kmax + 1):
                            ksl = slice(kc * P, (kc + 1) * P)
                            r = kc - cpq * j
                            qo = r * P if r > 0 else 0
                            quad = quad_pool.tile([P, 2, qg], FQ,
                                                  name="quad", tag="quad")
                            for ci in range(2):
                                h = 2 * pi + ci
                                hp = slice(32 * h, 32 * h + 32)
                                nc.tensor.matmul(
                                    quad[:, ci, qo:], kT[b][hp, ksl],
                                    qT[b][hp, j * qg + qo:(j + 1) * qg],
                                    start=True, stop=True,
                                    tile_position=(32 * h, 0))
                            et = exp_pool.tile([P, 2, qg], F16, name="et",
                                               tag="et")
                            if exp_mode == "flat" and qo == 0:
                                nc.scalar.activation(
                                    et[:].rearrange("p a b -> p (a b)"),
                                    quad[:].rearrange("p a b -> p (a b)"),
                                    mybir.ActivationFunctionType.Exp,
                                    scale=scale)
                            elif exp_mode in ("2ci", "flat"):
                                for ci in range(2):
                                    nc.scalar.activation(
                                        et[:, ci, qo:], quad[:, ci, qo:],
                                        mybir.ActivationFunctionType.Exp,
                                        scale=scale)
                            else:
                                nc.scalar.activation(
                                    et[:, :, qo:], quad[:, :, qo:],
                                    mybir.ActivationFunctionType.Exp,
                                    scale=scale)
                            if r >= 0:
                                blk = slice(r * P, (r + 1) * P)
                                nc.vector.tensor_tensor(
                                    et[:, :, blk], et[:, :, blk],
                                    bmask[:, None, :].to_broadcast((P, 2, P)),
                                    mybir.AluOpType.mult)
                            if prev is not None:
                                emit_av(*prev)
                                pump()
                            prev = (kc, et, qo)
                        emit_av(*prev)
                        pump()
                        flush_other()
                        run_or_defer(tail_items(b, j, pi, y_p),
                                     pend_other, defer_tail)
                    flush_slab()
                flush_other()
    nc.compile()
    return nc


_NC_CACHE = {}

# shipped configuration: v4 pipelined builder (fp16 attention operands,
# deferred normalize/projection + slab-prefetched phase A trickled into the
# attention kc loop, merged single-matmul output projection).  Measured
# ~282us vs ~362us for the v1 builder under interleaved min-wall timing
# (wall-clock carries +-100us of RPC/throttle noise per sample; only
# min-wall-per-side comparisons over many interleaved rounds are valid --
# see bench.py).  rel_err vs fp32 reference ~5.9e-4 (gate 2e-2).
CONFIG = {"quad_dt": "f32", "quad_bufs": 2, "y_bufs": 2, "aux_bufs": 2,
          "host_xt": True, "exp_bufs": 8}


def _get_nc(bpc=BPC, t=T, loop_n=0):
    key = (bpc, t, loop_n)
    if key not in _NC_CACHE:
        _NC_CACHE[key] = build_attention_v4(bpc=bpc, t=t, loop_n=loop_n,
                                            **CONFIG)
    return _NC_CACHE[key]


def shard_inputs(x, w_attn, w_proj, host_xt=None):
    """Host-side input marshalling: shard over batch; optionally pre-
    transpose x to [b, D, T] fp16 (the layout the device needs anyway)."""
    if host_xt is None:
        host_xt = CONFIG.get("host_xt", False)
    x = np.ascontiguousarray(np.asarray(x), dtype=np.float32)
    w_attn = np.ascontiguousarray(np.asarray(w_attn), dtype=np.float32)
    w_proj = np.ascontiguousarray(np.asarray(w_proj), dtype=np.float32)
    assert x.shape == (B, T, D), x.shape
    if host_xt:
        x = np.ascontiguousarray(
            x.transpose(0, 2, 1).astype(np.float16))
    return [
        {"x": x[c * BPC:(c + 1) * BPC], "w_attn": w_attn, "w_proj": w_proj}
        for c in range(N_CORES)
    ]


def _run(x, w_attn, w_proj, **spmd_kwargs):
    nc = _get_nc()
    in_maps = shard_inputs(x, w_attn, w_proj)
    res = run_bass_kernel_spmd(nc, in_maps, list(range(N_CORES)),
                               **spmd_kwargs)
    out = np.concatenate([res.results[c]["out"] for c in range(N_CORES)],
                         axis=0)
    return out.astype(np.float32), res


def kernel(x, w_attn, w_proj):
    out, _ = _run(x, w_attn, w_proj)
    return out


if __name__ == "__main__":
    nc = build_attention_nc()
    print("built ok")

